# revision 1
# baseline (speedup 1.0000x reference)
"""Trainium2 Bass kernel for nn_MultiHeadSelfAttention_65025804862080.

Full inputs in, full output out. Internally shards across 8 NeuronCores:
core i handles batch b=i//4 and heads {4j..4j+3} (j=i%4). Because the
reference reshapes [b, t, h*d] -> [b, h, t, d] directly (no transpose),
head h's Q/K/V derive from rows [128h, 128h+128) of the projection of x,
so the QKV work shards across cores with zero duplication. The output
projection is row-parallel: each core produces a [2048, 1024] partial sum
over its 4 heads; the host sums the 4 partials per batch and adds bo.

RoPE in the reference is indexed at the single position t=2048 (a faithful
replication of the original module's bug), making it position-independent;
the rotation therefore folds exactly into wq/wk (and bq/bk) on the host.

Per-head token permutation: head-local token t = tt*16 + m (tt = row
within the head's 128-row x block, m = feature chunk), stored as
pi = w*1024 + mp*128 + tt with m = 2*mp + w. Scores are computed
transposed (S^T: k on partitions, q free) so softmax exp output P^T
feeds the PV matmul directly with no transposes; softmax skips the max
subtraction (logits are ~N(0,1)); row sums come from a ones column
appended to V inside the PV matmul; normalization happens at the end
via reciprocal + a K=1 broadcast matmul.

Heads are packed in pairs on the partition axis (head 2a at partitions
0-63, head 2a+1 at 64-127) so matmul lhsT/rhs partition bases always
match (bass requires equality).
"""

import sys

if "/opt/trn_rl_repo" not in sys.path:
    sys.path.insert(0, "/opt/trn_rl_repo")

from contextlib import ExitStack

import ml_dtypes
import numpy as np

import concourse.tile as tile
from concourse import bacc, mybir
from concourse.bass_utils import run_bass_kernel_spmd

# Problem constants (hardcoded per the self-contained contract).
B, T, DM, H, DH = 2, 2048, 1024, 16, 64
N_CORES = 8
HPC = 4          # heads per core
RB = 512         # x-row block per core
F32 = mybir.dt.float32
F32R = mybir.dt.float32r
BF16 = mybir.dt.bfloat16
EXPF = mybir.ActivationFunctionType.Exp

# matmul input dtype: float32r streams fp32 data at 1 cycle/row (N>=256)
MM_DT = F32R


def _mm(x):
    """Matmul operands are already float32r-typed tiles; no-op."""
    return x


def build_program():
    """Build and compile the per-core SPMD Bass program. Same program for
    all 8 cores; per-core data differs only."""
    nc = bacc.Bacc(
        "TRN2", target_bir_lowering=False, debug=False, num_devices=N_CORES
    )

    xT = nc.dram_tensor("xT", [DM, RB], BF16, kind="ExternalInput").ap()
    wqT = nc.dram_tensor("wqT", [DM, DM], BF16, kind="ExternalInput").ap()
    wkT = nc.dram_tensor("wkT", [DM, DM], BF16, kind="ExternalInput").ap()
    wvT = nc.dram_tensor("wvT", [DM, DM], BF16, kind="ExternalInput").ap()
    woT4 = nc.dram_tensor("woT4", [HPC, DH, DM], BF16, kind="ExternalInput").ap()
    bqr = nc.dram_tensor("bqr", [1, DM], F32R, kind="ExternalInput").ap()
    bkr = nc.dram_tensor("bkr", [1, DM], F32R, kind="ExternalInput").ap()
    bvv = nc.dram_tensor("bv", [1, DM], F32R, kind="ExternalInput").ap()
    ones1 = nc.dram_tensor("ones1", [1, 512], F32R, kind="ExternalInput").ap()
    out = nc.dram_tensor("out", [T, DM], BF16, kind="ExternalOutput").ap()
    rcd = [
        nc.dram_tensor(f"rcd{h}", [1, 1024], mybir.dt.bfloat16).ap()
        for h in range(2)
    ]

    with tile.TileContext(nc) as tc:
        _emit(nc, tc, xT, wqT, wkT, wvT, woT4, bqr, bkr, bvv, ones1, out, rcd)

    nc.compile()
    return nc


def _emit(nc, tc, xT, wqT, wkT, wvT, woT4, bqr, bkr, bvv, ones1, out, rcd):
    ctx = ExitStack()
    with ctx:
        singles = ctx.enter_context(tc.tile_pool(name="singles", bufs=1))
        big = ctx.enter_context(tc.tile_pool(name="big", bufs=1))
        w_pool = ctx.enter_context(tc.tile_pool(name="wts", bufs=10))
        pt_pool = ctx.enter_context(tc.tile_pool(name="pt", bufs=6))
        stga_pool = ctx.enter_context(tc.tile_pool(name="stga", bufs=4))
        stgc_pool = ctx.enter_context(tc.tile_pool(name="stgc", bufs=6))
        bc_pool = ctx.enter_context(tc.tile_pool(name="bcp", bufs=4))

        import concourse.bass as bass_mod

        # --- load xT first: 8 tiles [128, 512] (feeds the first matmuls) ---
        XT = []
        for p in range(8):
            t = big.tile([128, RB], BF16, tag=f"xt{p}", name=f"xt{p}")
            for q2 in range(2):
                nc.sync.dma_start(
                    out=t[64 * q2 : 64 * (q2 + 1), :],
                    in_=xT[128 * p + 64 * q2 : 128 * p + 64 * (q2 + 1), :],
                )
            XT.append(t)

        # --- constants: allocate now, DMA later (so weight-chunk DMAs
        # lead the sync ring and the PE starts immediately) ---
        ones = singles.tile([128, 512], F32R, name="ones")
        ones_bcast = bass_mod.AP(
            tensor=ones1.tensor,
            offset=ones1.offset,
            ap=[[0, 128], list(ones1.ap[-1])],
        )
        bq_sb = singles.tile([1, DM], F32R, tag="bq", name="bq_sb")
        bk_sb = singles.tile([1, DM], F32R, tag="bk", name="bk_sb")
        bv_sb = singles.tile([1, DM], F32R, tag="bv", name="bv_sb")

        WO4 = []
        for l in range(HPC):
            t = big.tile([DH, DM], BF16, tag=f"wo{l}", name=f"wo{l}")
            WO4.append(t)

        # Pair-packed attention operands. QQ[a]/KK[a]: [128, 2048];
        # head 2a at partitions 0-63, head 2a+1 at 64-127; free dim =
        # pi = w*1024 + mp*128 + tt.
        QQ = [big.tile([128, T], BF16, tag=f"qq{a}", name=f"qq{a}") for a in range(2)]
        KK = [big.tile([128, T], BF16, tag=f"kk{a}", name=f"kk{a}") for a in range(2)]
        # VN[l]: [128(tt), 16*65]; slab s = w*8+mp at cols [65s, 65s+64),
        # ones at col 65s+64.
        VN = [big.tile([128, 16 * 65], BF16, tag=f"vn{l}", name=f"vn{l}") for l in range(HPC)]
        ones_bf = singles.tile([128, 16], BF16, tag="onesbf", name="ones_bf")
        nc.vector.memset(ones_bf, 1.0)
        # ON4[l]: normalized O^T per head, [64, 2048] at base partition 0.
        ON4 = [
            big.tile([DH, T], BF16, tag=f"on{l}", name=f"on{l}")
            for l in range(HPC)
        ]
        # reciprocal row staging (row 64 only is used)
        rc = big.tile([65, T], F32, tag="rc", name="rc")
        OU = [
            big.tile([DH, T], BF16, tag=f"ou{l}", name=f"ou{l}")
            for l in range(HPC)
        ]
        PO0 = big.tile([128, 16 * DM], BF16, tag="po0", name="PO0")
        RSM = [big.tile([128, 8], F32, tag=f"rsm{h}", name=f"rsm{h}") for h in range(2)]
        RSM2 = [big.tile([128, 8], F32, tag=f"rsm2{h}", name=f"rsm2{h}") for h in range(2)]
        RSR = [big.tile([128, 8], BF16, tag=f"rsr{h}", name=f"rsr{h}") for h in range(2)]

        # ---------------- Stage A: QKV projections ----------------
        with tc.tile_pool(name="psA", bufs=1, space="PSUM") as psA:
            # --- Q and K: transposed layout. out tile m2 = features
            # [128 m2, 128 m2 + 128) x 512 block-tokens. Feature
            # c = m*64+dd; tile m2 rows 0:64 = slab (w=0, mp=m2),
            # rows 64:128 = slab (w=1, mp=m2). ---
            for pname, wT, b_sb, dest in (
                ("q", wqT, bq_sb, QQ),
                ("k", wkT, bk_sb, KK),
            ):
                ps = [
                    psA.tile([128, RB], F32, tag=f"A{m2}", name=f"ps{pname}{m2}")
                    for m2 in range(8)
                ]
                for p in range(8):
                    wchunk = w_pool.tile([128, DM], BF16, tag="w", name=f"w{pname}{p}")
                    for q4 in range(4):
                        nc.sync.dma_start(
                            out=wchunk[32 * q4 : 32 * (q4 + 1), :],
                            in_=wT[128 * p + 32 * q4 : 128 * p + 32 * (q4 + 1), :],
                        )
                    for m2 in range(8):
                        nc.tensor.matmul(
                            ps[m2][:],
                            _mm(wchunk[:, 128 * m2 : 128 * (m2 + 1)]),
                            _mm(XT[p][:]),
                            start=(p == 0),
                            stop=False,
                        )
                if pname == "q":
                    nc.sync.dma_start(out=ones, in_=ones_bcast)
                    nc.sync.dma_start(out=bq_sb, in_=bqr)
                    nc.sync.dma_start(out=bk_sb, in_=bkr)
                    nc.sync.dma_start(out=bv_sb, in_=bvv)
                for m2 in range(8):
                    # bias: + b[c] * ones[r]  (K=1 matmul closes the group)
                    nc.tensor.matmul(
                        ps[m2][:],
                        _mm(b_sb[0:1, 128 * m2 : 128 * (m2 + 1)]),
                        _mm(ones[0:1, 0:RB]),
                        start=False,
                        stop=True,
                    )
                # drain via SBUF staging; same-partition-half pieces go by
                # DVE copy, cross-partition pieces by SBUF->SBUF DMA.
                for m2 in range(8):
                    stg = stga_pool.tile(
                        [128, RB], BF16, tag="stga", name=f"stg{pname}{m2}"
                    )
                    nc.vector.tensor_copy(stg[:], ps[m2][:])
                    for l in range(HPC):
                        a, z = l // 2, l % 2
                        dcols_w0 = slice(128 * m2, 128 * m2 + 128)
                        dcols_w1 = slice(1024 + 128 * m2, 1024 + 128 * m2 + 128)
                        scols = slice(128 * l, 128 * l + 128)
                        if z == 0:
                            nc.vector.tensor_copy(
                                dest[a][0:64, dcols_w0], stg[0:64, scols]
                            )
                            nc.gpsimd.dma_start(
                                out=dest[a][0:64, dcols_w1],
                                in_=stg[64:128, scols],
                            )
                        else:
                            nc.gpsimd.dma_start(
                                out=dest[a][64:128, dcols_w0],
                                in_=stg[0:64, scols],
                            )
                            nc.vector.tensor_copy(
                                dest[a][64:128, dcols_w1], stg[64:128, scols]
                            )

            # --- V: natural (token-major) layout. out tile (l, cc):
            # block-rows [128l, 128l+128) x features [512cc, 512cc+512) ---
            psv = [
                psA.tile([128, 512], F32, tag=f"A{l * 2 + cc}", name=f"psv{l}_{cc}")
                for l in range(HPC)
                for cc in range(2)
            ]
            for p in range(8):
                wchunk = w_pool.tile([128, DM], BF16, tag="w", name=f"wv{p}")
                for q4 in range(4):
                    nc.sync.dma_start(
                        out=wchunk[32 * q4 : 32 * (q4 + 1), :],
                        in_=wvT[128 * p + 32 * q4 : 128 * p + 32 * (q4 + 1), :],
                    )
                for l in range(HPC):
                    for cc in range(2):
                        nc.tensor.matmul(
                            psv[l * 2 + cc][:],
                            _mm(XT[p][:, 128 * l : 128 * (l + 1)]),
                            _mm(wchunk[:, 512 * cc : 512 * (cc + 1)]),
                            start=(p == 0),
                            stop=False,
                        )
            for l in range(HPC):
                for cc in range(2):
                    nc.tensor.matmul(
                        psv[l * 2 + cc][:],
                        _mm(ones[0:1, 0:128]),
                        _mm(bv_sb[0:1, 512 * cc : 512 * (cc + 1)]),
                        start=False,
                        stop=True,
                    )
            # drain V into the strided-65 layout: src col (mp, w, dd)
            # within chunk cc -> dst col (w*8 + 4cc + mp)*65 + dd.
            for l in range(HPC):
                vdst = VN[l].rearrange("p (w s8 e) -> p s8 w e", w=2, e=65)
                for cc in range(2):
                    src = psv[l * 2 + cc].rearrange(
                        "p (mp w e) -> p mp w e", mp=4, w=2
                    )
                    nc.vector.tensor_copy(
                        vdst[:, 4 * cc : 4 * cc + 4, :, 0:64], src
                    )

        for l in range(HPC):
            nc.sync.dma_start(out=WO4[l], in_=woT4[l])
            v3 = VN[l].rearrange("p (s e) -> p s e", e=65)
            nc.gpsimd.dma_start(out=v3[:, :, 64:65], in_=ones_bf[:])

        # ---------------- Stage B: attention per head ----------------
        with (
            tc.tile_pool(name="psS", bufs=2, space="PSUM") as psS,
            tc.tile_pool(name="psO", bufs=1, space="PSUM") as psO,
            tc.tile_pool(name="psC1", bufs=2, space="PSUM") as psC1,
        ):
            for l in range(HPC):
                a, z = l // 2, l % 2
                zs = slice(64 * z, 64 * z + 64)
                for hf in range(2):
                    op = psO.tile([65, 1024], F32, tag="o", name=f"po{l}_{hf}")

                    def emit_pv(pv):
                        pt_, s_ = pv
                        for qc in range(2):
                            nc.tensor.matmul(
                                op[:, 512 * qc : 512 * qc + 512],
                                _mm(VN[l][:, 65 * s_ : 65 * s_ + 65]),
                                _mm(pt_[:, 512 * qc : 512 * (qc + 1)]),
                                start=(s_ == 0),
                                stop=(s_ == 15),
                            )

                    # Software-pipelined: PV for slab n-1 issues between the
                    # S-matmuls of slab n and its exp, so the in-order PE
                    # never sits behind an exp it is waiting on.
                    prev = None
                    for s in range(16):
                        sp = psS.tile(
                            [128, 1024], F32, tag="s", name=f"ps{l}_{s}_{hf}"
                        )
                        for qc in range(2):
                            nc.tensor.matmul(
                                sp[:, 512 * qc : 512 * (qc + 1)],
                                _mm(KK[a][zs, 128 * s : 128 * s + 128]),
                                _mm(
                                    QQ[a][
                                        zs,
                                        1024 * hf
                                        + 512 * qc : 1024 * hf
                                        + 512 * qc
                                        + 512,
                                    ]
                                ),
                                start=True,
                                stop=True,
                            )
                        if prev is not None:
                            emit_pv(prev)
                        pt = pt_pool.tile(
                            [128, 1024], BF16, tag="pt", name=f"pt{l}_{s}_{hf}"
                        )
                        nc.scalar.activation(pt[:], sp[:], EXPF, scale=0.125)
                        prev = (pt, s)
                    emit_pv(prev)
                    # Drain this half promptly so the psO slot frees (keeps
                    # PE warm): O rows -> OU (bf16, DVE), sums -> rc (DVE).
                    cols = slice(1024 * hf, 1024 * (hf + 1))
                    nc.vector.tensor_copy(OU[l][:, cols], op[0:64, :])
                    nc.vector.tensor_copy(rc[64:65, cols], op[64:65, :])
                    # Per-half normalize chain (hf=0 hides under hf=1's
                    # slab loop): scatter sums row to [128, 8], reciprocal,
                    # gather to a DRAM row, broadcast, multiply.
                    nc.gpsimd.dma_start(out=RSM[hf][:], in_=rc[64:65, cols])
                    nc.vector.reciprocal(RSM2[hf][:], RSM[hf][:])
                    nc.vector.tensor_copy(RSR[hf][:], RSM2[hf][:])
                    nc.gpsimd.dma_start(out=rcd[hf], in_=RSR[hf][:])
                    bch = bc_pool.tile(
                        [64, 1024], BF16, tag="bcs", name=f"bch{l}_{hf}"
                    )
                    bcast_ap = bass_mod.AP(
                        tensor=rcd[hf].tensor,
                        offset=rcd[hf].offset,
                        ap=[[0, 64], list(rcd[hf].ap[-1])],
                    )
                    nc.gpsimd.dma_start(out=bch[:], in_=bcast_ap)
                    nc.vector.tensor_mul(
                        ON4[l][:, cols], OU[l][:, cols], bch[:]
                    )

                if l == 1:
                    # Window-1 output projection: heads 0-1 partials into
                    # PO0 while heads 2-3 attention keeps ACT busy.
                    for sq in range(16):
                        for cc in range(2):
                            oc1 = psC1.tile(
                                [128, 512], F32, tag="c1", name=f"oc1_{sq}_{cc}"
                            )
                            for ll in range(2):
                                nc.tensor.matmul(
                                    oc1[:],
                                    _mm(ON4[ll][:, 128 * sq : 128 * sq + 128]),
                                    _mm(WO4[ll][:, 512 * cc : 512 * (cc + 1)]),
                                    start=(ll == 0),
                                    stop=(ll == 1),
                                )
                            nc.vector.tensor_copy(
                                PO0[:, 1024 * sq + 512 * cc : 1024 * sq + 512 * cc + 512],
                                oc1[:],
                            )

        # ---------------- Stage C: output projection ----------------
        # Output is written slab-contiguously (row pi = sq*128 + tt);
        # the host un-permutes rows (t = tt*16 + 2*(sq%8) + sq//8).
        with tc.tile_pool(name="psC", bufs=3, space="PSUM") as psC:
            for sq in range(16):
                w_, mp_ = sq // 8, sq % 8
                oc = psC.tile([128, DM], F32, tag="c", name=f"oc{sq}")
                for cc in range(2):
                    for l in (2, 3):
                        nc.tensor.matmul(
                            oc[:, 512 * cc : 512 * (cc + 1)],
                            _mm(ON4[l][:, 128 * sq : 128 * sq + 128]),
                            _mm(WO4[l][:, 512 * cc : 512 * (cc + 1)]),
                            start=(l == 2),
                            stop=(l == 3),
                        )
                stgc = stgc_pool.tile([128, DM], BF16, tag="stgc", name=f"stgc{sq}")
                nc.vector.tensor_add(
                    stgc[:], oc[:], PO0[:, 1024 * sq : 1024 * (sq + 1)]
                )
                for q4 in range(4):
                    # tail has no other DMA traffic: use both rings
                    (nc.gpsimd if q4 % 2 else nc.sync).dma_start(
                        out=out[128 * sq + 32 * q4 : 128 * sq + 32 * (q4 + 1), :],
                        in_=stgc[32 * q4 : 32 * (q4 + 1), :],
                    )


_NC_CACHE = None


def _get_program():
    global _NC_CACHE
    if _NC_CACHE is None:
        _NC_CACHE = build_program()
    return _NC_CACHE


def _prep_host(x, wq, bq, wk, bk, wv, bv, wo, bo, cos, sin):
    """Host-side shard prep: RoPE folding, transposes, per-core slicing."""
    f32 = np.float32
    x = np.asarray(x, f32)
    wq, wk, wv, wo = (np.asarray(a, f32) for a in (wq, wk, wv, wo))
    bq, bk, bv, bo = (np.asarray(a, f32) for a in (bq, bk, bv, bo))
    cos, sin = np.asarray(cos, f32), np.asarray(sin, f32)

    # RoPE at fixed position T (reference bug, replicated): fold into weights.
    c_row = cos[T]  # [64]
    s_row = sin[T]  # [64]
    Cv = np.tile(c_row, H)  # [1024]
    Sv = np.tile(s_row, H)  # [1024]
    sgn = np.where(np.arange(DM) % 2 == 0, -1.0, 1.0).astype(f32)
    Ss = (sgn * Sv).astype(f32)
    swap = np.arange(DM) ^ 1  # adjacent-pair swap

    wq_rot = Cv[:, None] * wq + Ss[:, None] * wq[swap, :]
    wk_rot = Cv[:, None] * wk + Ss[:, None] * wk[swap, :]
    bq_rot = Cv * bq + Ss * bq[swap]
    bk_rot = Cv * bk + Ss * bk[swap]

    bf = ml_dtypes.bfloat16
    wqT = np.ascontiguousarray(wq_rot.T).astype(bf)
    wkT = np.ascontiguousarray(wk_rot.T).astype(bf)
    wvT = np.ascontiguousarray(wv.T).astype(bf)

    in_maps = []
    for i in range(N_CORES):
        b, j = i // 4, i % 4
        xTc = np.ascontiguousarray(x[b, RB * j : RB * (j + 1), :].T).astype(ml_dtypes.bfloat16)
        woT4 = np.stack(
            [
                np.ascontiguousarray(wo[:, DH * (4 * j + l) : DH * (4 * j + l + 1)].T)
                for l in range(HPC)
            ]
        ).astype(ml_dtypes.bfloat16)
        in_maps.append(
            {
                "xT": xTc,
                "wqT": wqT,
                "wkT": wkT,
                "wvT": wvT,
                "woT4": woT4,
                "bqr": bq_rot.reshape(1, DM),
                "bkr": bk_rot.reshape(1, DM),
                "bv": bv.reshape(1, DM),
                "ones1": np.ones((1, 512), f32),
            }
        )
    return in_maps, bo


def kernel(x, wq, bq, wk, bk, wv, bv, wo, bo, cos, sin, _trace=False, _trace_kwargs=None):
    nc = _get_program()
    in_maps, bo_np = _prep_host(x, wq, bq, wk, bk, wv, bv, wo, bo, cos, sin)
    kw = {}
    if _trace:
        kw["trace"] = True
        if _trace_kwargs:
            kw.update(_trace_kwargs)
    res = run_bass_kernel_spmd(nc, in_maps, core_ids=list(range(N_CORES)), **kw)
    out = np.zeros((B, T, DM), np.float32)
    for i in range(N_CORES):
        # rows arrive as pi = (w*8+mp)*128 + tt; t = tt*16 + 2*mp + w
        part = res.results[i]["out"].astype(np.float32)
        part = (
            part.reshape(2, 8, 128, DM).transpose(2, 1, 0, 3).reshape(T, DM)
        )
        out[i // 4] += part
    out += bo_np[None, None, :]
    kernel.last_results = res
    return out



# revision 2
# speedup vs baseline: 1.0222x; 1.0222x over previous
"""Trainium2 Bass kernel for nn_MultiHeadSelfAttention_65025804862080 (v2).

Full inputs in, full output out. Core i handles batch b=i//4 and heads
{4j..4j+3} (j=i%4). The reference's no-transpose head split means head h's
Q/K/V derive from x tokens [128h, 128h+128) only, so QKV shards across
cores with zero duplication. RoPE is position-independent (reference
indexes cos/sin at the single position t=T) and folds into wq/wk on host.

v2 schedule (vs baseline): big DMA descriptors spread across the sync +
scalar HWDGE rings (startup latency was 23us); QKV projections interleave
PSUM half-groups so drains overlap matmuls; each weight chunk is loaded
once (V chunks shared with the deferred heads-2/3 projection); Q/K bias
folds into the DVE drain (tensor_scalar); V for heads 2-3 and the whole
output projection run as per-slab filler matmuls inside the attention
loop so the PE never idles (p-state stays at 2.4GHz) while ACT streams
the exps; output projection is K=128 (head pairs stacked on partitions)
accumulating both pairs in PSUM; softmax normalization is a DVE
reciprocal row + DRAM-bounce broadcast + 2x-mode DVE multiply, fully off
the PE critical path.

Per-head token permutation: pi = w*1024 + mp*128 + tt where the head's
feature chunk m = 2*mp + w and tt = token row within the head's 128-row
x block. Scores are computed transposed (S^T: k on partitions) so the exp
output P^T feeds PV directly; row sums come from a ones column appended
to V (M=65); exp skips max subtraction (logits ~N(0,1) * 1/8).
"""

import sys

if "/opt/trn_rl_repo" not in sys.path:
    sys.path.insert(0, "/opt/trn_rl_repo")

from contextlib import ExitStack

import ml_dtypes
import numpy as np

import concourse.tile as tile
from concourse import bacc, mybir
from concourse.bass_utils import run_bass_kernel_spmd

B, T, DM, H, DH = 2, 2048, 1024, 16, 64
N_CORES = 8
HPC = 4          # heads per core
RB = 512         # x-row block per core
F32 = mybir.dt.float32
F32R = mybir.dt.float32r
BF16 = mybir.dt.bfloat16
EXPF = mybir.ActivationFunctionType.Exp
ADD = mybir.AluOpType.add


def build_program():
    nc = bacc.Bacc(
        "TRN2", target_bir_lowering=False, debug=False, num_devices=N_CORES
    )

    # xTp: p-major packed x^T block: xTp[r, 512*p + c] = xT[128p + r, c]
    xTp = nc.dram_tensor("xTp", [128, 8 * RB], BF16, kind="ExternalInput").ap()
    wqT = nc.dram_tensor("wqT", [8, 128, DM], BF16, kind="ExternalInput").ap()
    wkT = nc.dram_tensor("wkT", [8, 128, DM], BF16, kind="ExternalInput").ap()
    wvT = nc.dram_tensor("wvT", [8, 128, DM], BF16, kind="ExternalInput").ap()
    wop = nc.dram_tensor("wop", [2, 128, DM], BF16, kind="ExternalInput").ap()
    bqp = nc.dram_tensor("bqp", [128, 8], F32, kind="ExternalInput").ap()
    bkp = nc.dram_tensor("bkp", [128, 8], F32, kind="ExternalInput").ap()
    bvr = nc.dram_tensor("bvr", [1, DM], F32R, kind="ExternalInput").ap()
    ones1 = nc.dram_tensor("ones1", [1, 128], F32R, kind="ExternalInput").ap()
    out = nc.dram_tensor("out", [T, DM], BF16, kind="ExternalOutput").ap()
    rcd = [nc.dram_tensor(f"rcd{i}", [1, DM], BF16).ap() for i in range(2)]

    with tile.TileContext(nc) as tc:
        _emit(nc, tc, xTp, wqT, wkT, wvT, wop, bqp, bkp, bvr, ones1, out, rcd)

    nc.compile()
    return nc


def _emit(nc, tc, xTp, wqT, wkT, wvT, wop, bqp, bkp, bvr, ones1, out, rcd):
    import concourse.bass as bass_mod

    ctx = ExitStack()
    with ctx:
        singles = ctx.enter_context(tc.tile_pool(name="singles", bufs=1))
        big = ctx.enter_context(tc.tile_pool(name="big", bufs=1))
        w_pool = ctx.enter_context(tc.tile_pool(name="wts", bufs=24))
        pt_pool = ctx.enter_context(tc.tile_pool(name="pt", bufs=4))
        stg_pool = ctx.enter_context(tc.tile_pool(name="stg", bufs=4))
        stgc_pool = ctx.enter_context(tc.tile_pool(name="stgc", bufs=4))
        bch_pool = ctx.enter_context(tc.tile_pool(name="bch", bufs=2))

        # ---- big persistent tiles ----
        XT = big.tile([128, 8 * RB], BF16, tag="xt", name="XT")
        # QQ/KK: pair-major free dim: col = a*2048 + pi, pi = w*1024+mp*128+tt
        # partitions: z*64 + dd (z = head parity within pair)
        QQ = big.tile([128, 2 * T], BF16, tag="qq", name="QQ")
        KK = big.tile([128, 2 * T], BF16, tag="kk", name="KK")
        # VN[l]: [128(tt), 16*65]; slab s = w*8 + mp_global; ones at col 65s+64
        VN = [big.tile([128, 16 * 65], BF16, tag=f"vn{l}", name=f"vn{l}")
              for l in range(HPC)]
        # ONP[a]: normalized O^T, pair-stacked: partition z*64+dd, free pi
        ONP = [big.tile([128, T], BF16, tag=f"onp{a}", name=f"onp{a}")
               for a in range(2)]
        WOP = [big.tile([128, DM], BF16, tag=f"wop{a}", name=f"wopp{a}")
               for a in range(2)]
        OU = big.tile([DH, 2 * DM], BF16, tag="ou", name="OU")  # per-unit O^T
        ON1 = big.tile([DH, DM], BF16, tag="on1", name="ON1")  # z=1 scratch
        rcp = big.tile([1, 2 * DM], BF16, tag="rcp", name="rcp")

        bq_sb = singles.tile([128, 8], F32, tag="bq", name="bq_sb")
        bk_sb = singles.tile([128, 8], F32, tag="bk", name="bk_sb")
        bv_sb = singles.tile([1, DM], F32R, tag="bv", name="bv_sb")
        ones_r = singles.tile([1, 128], F32R, tag="ones", name="ones_r")
        onescol = singles.tile([128, 16], BF16, tag="onescol", name="onescol")
        ones64 = singles.tile([1, 64], BF16, tag="ones64", name="ones64")
        nc.vector.memset(onescol, 1.0)
        nc.vector.memset(ones64, 1.0)

        # ---- initial DMAs ----
        # One descriptor lands on ONE of the 16 DMA engines (~40GB/s each),
        # so startup-critical tiles are split into several descriptors that
        # fan out round-robin; everything else is one descriptor per chunk,
        # all triggered up front (descriptors self-gate on semaphores).
        # Ring budget matters: each trigger is ~600ns of sequencer time.
        x3o = XT.rearrange("p (k c) -> p k c", c=RB)
        x3i = xTp.rearrange("p (k c) -> p k c", c=RB)
        W = {}
        for nm, src in (("q", wqT), ("k", wkT), ("v", wvT)):
            for p in range(8):
                W[(nm, p)] = w_pool.tile([128, DM], BF16, tag="w",
                                         name=f"w{nm}{p}")

        # x on the sync ring, wq on the scalar ring IN PARALLEL so the
        # first matmul's two operands land together (~3us after preamble)
        for p in range(8):
            nsplit = 2 if p < 2 else 1
            for h in range(nsplit):
                w = 128 // nsplit
                nc.sync.dma_start(out=x3o[w * h:w * (h + 1), p, :],
                                  in_=x3i[w * h:w * (h + 1), p, :])
        nc.scalar.dma_start(out=bq_sb, in_=bqp)
        nc.scalar.dma_start(out=bk_sb, in_=bkp)
        nc.scalar.dma_start(out=bv_sb, in_=bvr)
        nc.scalar.dma_start(out=ones_r, in_=ones1)
        for p in range(8):
            nsplit = 4 if p == 0 else 2
            for h in range(nsplit):
                w = 128 // nsplit
                nc.scalar.dma_start(
                    out=W[("q", p)][w * h:w * (h + 1), :],
                    in_=wqT[p, w * h:w * (h + 1), :])
        # K and V both on scalar: the sync ring must stay clear for the
        # just-in-time Q/K drain DMAs
        for p in range(8):
            for h in range(2):
                nc.scalar.dma_start(out=W[("k", p)][64 * h:64 * (h + 1), :],
                                    in_=wkT[p, 64 * h:64 * (h + 1), :])
        for p in range(8):
            nc.scalar.dma_start(out=W[("v", p)], in_=wvT[p])

        # ---------- Stage A: QKV projections (interleaved halves) ----------
        def qk_drain(ps, m2, b_sb, dest):
            """Drain one Q/K psum m2-tile into QQ/KK pair layout + bias.
            All on DVE: ACT's in-order queue would serialize these behind
            each half's last matmul and starve the first exps."""
            src_e = ps[0:64, :].rearrange("p (a zz x) -> p zz a x", a=2, zz=2)
            src_o = ps[64:128, :].rearrange("p (a zz x) -> p zz a x", a=2, zz=2)
            d3 = dest.rearrange("p (a q) -> p a q", a=2)
            # z0w0: psum rows 0:64 of even-parity heads -> same partitions
            nc.vector.tensor_scalar(
                out=d3[0:64, :, 128 * m2:128 * m2 + 128],
                in0=src_e[:, 0, :, :], scalar1=b_sb[0:64, m2:m2 + 1],
                scalar2=None, op0=ADD)
            # z1w1: psum rows 64:128 of odd-parity heads -> same partitions
            nc.vector.tensor_scalar(
                out=d3[64:128, :, 1024 + 128 * m2:1024 + 128 * m2 + 128],
                in0=src_o[:, 1, :, :], scalar1=b_sb[64:128, m2:m2 + 1],
                scalar2=None, op0=ADD)
            # cross-partition pieces via stg + SBUF->SBUF DMA:
            # (z1,w0) = psum rows 0:64 of odd-parity heads -> dest rows 64:128
            # (z0,w1) = psum rows 64:128 of even-parity heads -> dest rows 0:64
            stg = stg_pool.tile([128, 256], BF16, tag="stg", name=f"stg{m2}")
            stg3 = stg.rearrange("p (a x) -> p a x", a=2)
            nc.vector.tensor_scalar(
                out=stg3[0:64], in0=src_e[:, 1, :, :],
                scalar1=b_sb[0:64, m2:m2 + 1], scalar2=None, op0=ADD)
            nc.vector.tensor_scalar(
                out=stg3[64:128], in0=src_o[:, 0, :, :],
                scalar1=b_sb[64:128, m2:m2 + 1], scalar2=None, op0=ADD)
            # cross DMAs ride the sync HWDGE ring just-in-time (the gpsimd
            # SWDGE queue serializes at ~1us/trigger and delayed QQ/KK)
            # z1w0: dest[64:128, a*2048 + m2*128] <- stg rows 0:64
            nc.sync.dma_start(
                out=d3[64:128, :, 128 * m2:128 * m2 + 128], in_=stg3[0:64])
            # z0w1: dest[0:64, a*2048 + 1024 + m2*128] <- stg rows 64:128
            nc.sync.dma_start(
                out=d3[0:64, :, 1024 + 128 * m2:1024 + 128 * m2 + 128],
                in_=stg3[64:128])

        def v_drain(ps, l, cc):
            vdst = VN[l].rearrange("p (w s8 e) -> p s8 w e", w=2, e=65)
            src = ps.rearrange("p (mp w e) -> p mp w e", mp=4, w=2)
            nc.vector.tensor_copy(vdst[:, 4 * cc:4 * cc + 4, :, 0:64], src)
            if cc == 1:
                v3 = VN[l].rearrange("p (s e) -> p s e", e=65)
                nc.vector.tensor_copy(
                    v3[:, :, 64:65], onescol.rearrange("p (s o) -> p s o", o=1))

        with tc.tile_pool(name="psA", bufs=1, space="PSUM") as psA:
            psq = [psA.tile([128, RB], F32, tag=f"A{i}", name=f"psq{i}")
                   for i in range(8)]
            for pname, b_sb, dest in (("q", bq_sb, QQ), ("k", bk_sb, KK)):
                for half in range(2):
                    m2s = range(4 * half, 4 * half + 4)
                    for p in range(8):
                        for m2 in m2s:
                            nc.tensor.matmul(
                                psq[m2][:],
                                W[(pname, p)][:, 128 * m2:128 * (m2 + 1)],
                                x3o[:, p, :],
                                start=(p == 0), stop=(p == 7))
                    # drain this half while the other half's matmuls stream
                    for m2 in m2s:
                        qk_drain(psq[m2], m2, b_sb, dest)
            # V head 0 only; heads 1-3 run as fillers inside attention
            psv = {}
            for cc in range(2):
                psv[(0, cc)] = psA.tile(
                    [128, 512], F32, tag=f"A{cc * 2}", name=f"psv0_{cc}")
            for p in range(8):
                for cc in range(2):
                    nc.tensor.matmul(
                        psv[(0, cc)][:],
                        x3o[:, p, 0:128],
                        W[("v", p)][:, 512 * cc:512 * (cc + 1)],
                        start=(p == 0), stop=False)
            for cc in range(2):
                nc.tensor.matmul(
                    psv[(0, cc)][:], ones_r[0:1, 0:128],
                    bv_sb[0:1, 512 * cc:512 * (cc + 1)],
                    start=False, stop=True)
                v_drain(psv[(0, cc)], 0, cc)

        # WOP is first needed by the pair-1 output projection; load it late
        # so it never delays the startup-critical x/weight descriptors.
        nc.sync.dma_start(out=WOP[0], in_=wop[0])
        nc.scalar.dma_start(out=WOP[1], in_=wop[1])

        # ---------- Stage B: flat software-pipelined attention ----------
        # 128 global slab-cycles (8 units x 16 slabs). Per cycle gs:
        #   ACT: exp(gs)            (paced back-to-back, the phase pacer)
        #   PE:  S(gs+2), PV(gs-1), [<=1 filler]
        # S leads its exp by 2 cycles and PV lags its exp by 1, so every PE
        # gate (psS WAR on exp(gs), pt RAW on exp(gs-1)) cleared >=1 cycle
        # before execution: the PE never micro-stalls (keeps the 2.4GHz
        # p-state), and the ACT stream never waits on S.
        fillers = []  # queue of closures, each emitting ~1-2 PE matmuls

        def make_v_fillers(psC, l):
            tiles = {}

            def mk(cc, p):
                def go():
                    if p == 0:
                        tiles[cc] = psC.tile(
                            [128, 512], F32, tag="c", name=f"psv{l}_{cc}")
                    nc.tensor.matmul(
                        tiles[cc][:],
                        x3o[:, p, 128 * l:128 * (l + 1)],
                        W[("v", p)][:, 512 * cc:512 * (cc + 1)],
                        start=(p == 0), stop=False)
                    if p == 7:
                        nc.tensor.matmul(
                            tiles[cc][:], ones_r[0:1, 0:128],
                            bv_sb[0:1, 512 * cc:512 * (cc + 1)],
                            start=False, stop=True)
                        v_drain(tiles[cc], l, cc)
                return go

            for cc in range(2):
                for p in range(8):
                    fillers.append(mk(cc, p))

        def make_outproj_fillers(psC, sqs, tail=False):
            def mk(sq, cc, use_act):
                def go():
                    oc = psC.tile([128, 512], F32, tag="c",
                                  name=f"oc{sq}_{cc}")
                    for a in range(2):
                        nc.tensor.matmul(
                            oc[:],
                            ONP[a][:, 128 * sq:128 * sq + 128],
                            WOP[a][:, 512 * cc:512 * (cc + 1)],
                            start=(a == 0), stop=(a == 1))
                    stgc = stgc_pool.tile([128, 512], BF16, tag="stgc",
                                          name=f"stgc{sq}_{cc}")
                    if use_act:  # ACT is idle in the tail; split the drain
                        nc.scalar.activation(
                            stgc[:], oc[:], mybir.ActivationFunctionType.Copy)
                    else:
                        nc.vector.tensor_copy(stgc[:], oc[:])
                    # never the scalar ring mid-attention (ACT sequencer)
                    eng = nc.sync if (sq + cc) % 2 == 0 else nc.gpsimd
                    eng.dma_start(
                        out=out[128 * sq:128 * (sq + 1),
                                512 * cc:512 * (cc + 1)],
                        in_=stgc[:])
                return go

            for sq in sqs:
                for cc in range(2):
                    fillers.append(mk(sq, cc, tail and (sq + cc) % 2 == 1))

        with (
            tc.tile_pool(name="psS", bufs=2, space="PSUM") as psS,
            tc.tile_pool(name="psO", bufs=1, space="PSUM") as psO,
            tc.tile_pool(name="psC", bufs=2, space="PSUM") as psC,
        ):
            AQ = QQ.rearrange("p (a q) -> p a q", a=2)
            AK = KK.rearrange("p (a q) -> p a q", a=2)
            # V heads 1-3 projections become the early fillers
            for l in (1, 2, 3):
                make_v_fillers(psC, l)
            # unit order: head-major for V-filler readiness; (3,*) before
            # the last unit so only z=0 (2,1) gates the tail
            units = [(0, 0), (0, 1), (1, 0), (1, 1),
                     (2, 0), (3, 0), (3, 1), (2, 1)]
            NU = len(units)
            sp_t = {}    # gs -> psS tile
            pt_t = {}    # gs -> exp output tile
            op_t = {}    # unit -> psO tile
            OUR = big.tile([65, 2 * DM], BF16, tag="our", name="OUR")

            def emit_S(gs):
                u, s = divmod(gs, 16)
                l, hf = units[u]
                a, z = l // 2, l % 2
                zs = slice(64 * z, 64 * z + 64)
                sp = psS.tile([128, 1024], F32, tag="s", name=f"ps{gs}")
                sp_t[gs] = sp
                for qc in range(2):
                    nc.tensor.matmul(
                        sp[:, 512 * qc:512 * (qc + 1)],
                        AK[zs, a, 128 * s:128 * s + 128],
                        AQ[zs, a, 1024 * hf + 512 * qc:
                           1024 * hf + 512 * qc + 512],
                        start=True, stop=True)

            def emit_exp(gs):
                pt = pt_pool.tile([128, 1024], BF16, tag="pt", name=f"pt{gs}")
                pt_t[gs] = pt
                nc.scalar.activation(pt[:], sp_t.pop(gs)[:], EXPF, scale=0.125)

            def emit_PV(gs):
                u, s = divmod(gs, 16)
                l, hf = units[u]
                if s == 0:
                    op_t[u] = psO.tile([65, 1024], F32, tag="o", name=f"po{u}")
                op = op_t[u]
                pt = pt_t.pop(gs)
                for qc in range(2):
                    nc.tensor.matmul(
                        op[:, 512 * qc:512 * qc + 512],
                        VN[l][:, 65 * s:65 * s + 65],
                        pt[:, 512 * qc:512 * (qc + 1)],
                        start=(s == 0), stop=(s == 15))
                if s == 15:
                    emit_norm(u)

            def emit_norm(u):
                l, hf = units[u]
                a, z = l // 2, l % 2
                op = op_t.pop(u)
                cols = slice(1024 * hf, 1024 * (hf + 1))
                # bank-wise [65,512] drains so the next unit's first PV
                # (one cycle later) finds the psO banks already free
                for qc in range(2):
                    nc.vector.tensor_copy(
                        OUR[:, 1024 * hf + 512 * qc:1024 * hf + 512 * (qc + 1)],
                        op[:, 512 * qc:512 * (qc + 1)])
                rc = OUR[64:65, cols]
                scr = stg_pool.tile([128, 8], BF16, tag="scr", name=f"sc{u}")
                scr2 = stg_pool.tile([128, 8], BF16, tag="sc2", name=f"s2{u}")
                # the last two units gate the tail: use the low-latency sync
                # HWDGE ring for their chain instead of SWDGE
                dring = nc.sync if u >= 6 else nc.gpsimd
                dring.dma_start(
                    out=scr, in_=rc.rearrange("o (r c) -> o r c", c=8))
                with nc.allow_low_precision(
                        reason="softmax denom reciprocal in bf16: ~0.4% "
                               "rel, well inside the 2e-2 gate"):
                    nc.vector.reciprocal(scr2, scr)
                dring.dma_start(out=rcd[u % 2], in_=scr2)
                bch = bch_pool.tile([64, DM], BF16, tag="b", name=f"bch{u}")
                bcast_ap = bass_mod.AP(
                    tensor=rcd[u % 2].tensor, offset=rcd[u % 2].offset,
                    ap=[[0, 64], list(rcd[u % 2].ap[-1])])
                dring.dma_start(out=bch, in_=bcast_ap)
                if z == 0:
                    nc.vector.tensor_mul(
                        ONP[a][0:64, cols], OUR[0:64, cols], bch[:])
                else:
                    # DVE cannot cross partitions: multiply into a base-0
                    # scratch, then DMA up to partitions 64-127 of ONP.
                    nc.vector.tensor_mul(ON1[:], OUR[0:64, cols], bch[:])
                    dring.dma_start(out=ONP[a][64:128, cols], in_=ON1[:])

            NG = 16 * NU
            emit_S(0)
            emit_S(1)
            for gs in range(NG):
                if gs == 106:
                    # sq<8 outproj: inputs complete once unit 5's normalize
                    # chain (~8us of DMA latency) lands
                    make_outproj_fillers(psC, range(8))
                emit_exp(gs)
                if gs + 2 < NG:
                    emit_S(gs + 2)
                if gs >= 2:
                    # PV lags its exp by 2 cycles so its pt-RAW semaphore
                    # landed a full cycle ago: no ACT->PE wait on PV
                    emit_PV(gs - 2)
                if fillers:
                    fillers.pop(0)()
            emit_PV(NG - 2)
            emit_PV(NG - 1)
            # tail: sq>=8 outproj (needs the last two units)
            make_outproj_fillers(psC, range(8, 16), tail=True)
            while fillers:
                fillers.pop(0)()


_NC_CACHE = None


def _get_program():
    global _NC_CACHE
    if _NC_CACHE is None:
        _NC_CACHE = build_program()
    return _NC_CACHE


def _prep_host(x, wq, bq, wk, bk, wv, bv, wo, bo, cos, sin):
    f32 = np.float32
    bf = ml_dtypes.bfloat16
    x = np.asarray(x, f32)
    wq, wk, wv, wo = (np.asarray(a, f32) for a in (wq, wk, wv, wo))
    bq, bk, bv, bo = (np.asarray(a, f32) for a in (bq, bk, bv, bo))
    cos, sin = np.asarray(cos, f32), np.asarray(sin, f32)

    # RoPE at fixed position T (reference bug, replicated): fold into weights.
    c_row = cos[T]
    s_row = sin[T]
    Cv = np.tile(c_row, H)
    Sv = np.tile(s_row, H)
    sgn = np.where(np.arange(DM) % 2 == 0, -1.0, 1.0).astype(f32)
    Ss = (sgn * Sv).astype(f32)
    swap = np.arange(DM) ^ 1

    wq_rot = Cv[:, None] * wq + Ss[:, None] * wq[swap, :]
    wk_rot = Cv[:, None] * wk + Ss[:, None] * wk[swap, :]
    bq_rot = Cv * bq + Ss * bq[swap]
    bk_rot = Cv * bk + Ss * bk[swap]

    wqTc = np.ascontiguousarray(wq_rot.T).reshape(8, 128, DM).astype(bf)
    wkTc = np.ascontiguousarray(wk_rot.T).reshape(8, 128, DM).astype(bf)
    wvTc = np.ascontiguousarray(wv.T).reshape(8, 128, DM).astype(bf)
    # bias per m2-tile partition: bqp[r, m2] = bq_rot[128*m2 + r]
    bqp = np.ascontiguousarray(bq_rot.reshape(8, 128).T).astype(f32)
    bkp = np.ascontiguousarray(bk_rot.reshape(8, 128).T).astype(f32)

    in_maps = []
    for i in range(N_CORES):
        b, j = i // 4, i % 4
        xT = x[b, RB * j:RB * (j + 1), :].T  # [1024, 512]
        xTp = np.ascontiguousarray(
            xT.reshape(8, 128, RB).transpose(1, 0, 2).reshape(128, 8 * RB)
        ).astype(bf)
        wopc = np.stack([
            np.ascontiguousarray(
                wo[:, 256 * j + 128 * a:256 * j + 128 * (a + 1)].T)
            for a in range(2)
        ]).astype(bf)
        in_maps.append({
            "xTp": xTp, "wqT": wqTc, "wkT": wkTc, "wvT": wvTc, "wop": wopc,
            "bqp": bqp, "bkp": bkp, "bvr": bv.reshape(1, DM),
            "ones1": np.ones((1, 128), f32),
        })
    return in_maps, bo


def kernel(x, wq, bq, wk, bk, wv, bv, wo, bo, cos, sin,
           _trace=False, _trace_kwargs=None):
    nc = _get_program()
    in_maps, bo_np = _prep_host(x, wq, bq, wk, bk, wv, bv, wo, bo, cos, sin)
    kw = {}
    if _trace:
        kw["trace"] = True
        if _trace_kwargs:
            kw.update(_trace_kwargs)
    res = run_bass_kernel_spmd(nc, in_maps, core_ids=list(range(N_CORES)), **kw)
    outf = np.zeros((B, T, DM), np.float32)
    for i in range(N_CORES):
        part = res.results[i]["out"].astype(np.float32)
        # rows arrive as pi = (w*8+mp)*128 + tt; t = tt*16 + 2*mp + w
        part = part.reshape(2, 8, 128, DM).transpose(2, 1, 0, 3).reshape(T, DM)
        outf[i // 4] += part
    outf += bo_np[None, None, :]
    kernel.last_results = res
    return outf


# revision 3
# speedup vs baseline: 1.0248x; 1.0026x over previous
"""Trainium2 Bass kernel for nn_MultiHeadSelfAttention_65025804862080 (v2).

Full inputs in, full output out. Core i handles batch b=i//4 and heads
{4j..4j+3} (j=i%4). The reference's no-transpose head split means head h's
Q/K/V derive from x tokens [128h, 128h+128) only, so QKV shards across
cores with zero duplication. RoPE is position-independent (reference
indexes cos/sin at the single position t=T) and folds into wq/wk on host.

v2 schedule (vs baseline): big DMA descriptors spread across the sync +
scalar HWDGE rings (startup latency was 23us); QKV projections interleave
PSUM half-groups so drains overlap matmuls; each weight chunk is loaded
once (V chunks shared with the deferred heads-2/3 projection); Q/K bias
folds into the DVE drain (tensor_scalar); V for heads 2-3 and the whole
output projection run as per-slab filler matmuls inside the attention
loop so the PE never idles (p-state stays at 2.4GHz) while ACT streams
the exps; output projection is K=128 (head pairs stacked on partitions)
accumulating both pairs in PSUM; softmax normalization is a DVE
reciprocal row + DRAM-bounce broadcast + 2x-mode DVE multiply, fully off
the PE critical path.

Per-head token permutation: pi = w*1024 + mp*128 + tt where the head's
feature chunk m = 2*mp + w and tt = token row within the head's 128-row
x block. Scores are computed transposed (S^T: k on partitions) so the exp
output P^T feeds PV directly; row sums come from a ones column appended
to V (M=65); exp skips max subtraction (logits ~N(0,1) * 1/8).
"""

import sys

if "/opt/trn_rl_repo" not in sys.path:
    sys.path.insert(0, "/opt/trn_rl_repo")

from contextlib import ExitStack

import ml_dtypes
import numpy as np

import concourse.tile as tile
from concourse import bacc, mybir
from concourse.bass_utils import run_bass_kernel_spmd

B, T, DM, H, DH = 2, 2048, 1024, 16, 64
N_CORES = 8
HPC = 4          # heads per core
RB = 512         # x-row block per core
F32 = mybir.dt.float32
F32R = mybir.dt.float32r
BF16 = mybir.dt.bfloat16
EXPF = mybir.ActivationFunctionType.Exp
ADD = mybir.AluOpType.add


def build_program():
    nc = bacc.Bacc(
        "TRN2", target_bir_lowering=False, debug=False, num_devices=N_CORES
    )

    # xTp: p-major packed x^T block: xTp[r, 512*p + c] = xT[128p + r, c]
    xTp = nc.dram_tensor("xTp", [128, 8 * RB], BF16, kind="ExternalInput").ap()
    wqT = nc.dram_tensor("wqT", [8, 128, DM], BF16, kind="ExternalInput").ap()
    wkT = nc.dram_tensor("wkT", [8, 128, DM], BF16, kind="ExternalInput").ap()
    wvT = nc.dram_tensor("wvT", [8, 128, DM], BF16, kind="ExternalInput").ap()
    wop = nc.dram_tensor("wop", [2, 128, DM], BF16, kind="ExternalInput").ap()
    bqp = nc.dram_tensor("bqp", [128, 8], F32, kind="ExternalInput").ap()
    bkp = nc.dram_tensor("bkp", [128, 8], F32, kind="ExternalInput").ap()
    bvr = nc.dram_tensor("bvr", [1, DM], F32R, kind="ExternalInput").ap()
    ones1 = nc.dram_tensor("ones1", [1, 128], F32R, kind="ExternalInput").ap()
    out = nc.dram_tensor("out", [T, DM], BF16, kind="ExternalOutput").ap()
    rcd = [nc.dram_tensor(f"rcd{i}", [1, DM], BF16).ap() for i in range(2)]

    with tile.TileContext(nc) as tc:
        _emit(nc, tc, xTp, wqT, wkT, wvT, wop, bqp, bkp, bvr, ones1, out, rcd)

    nc.compile()
    return nc


def _emit(nc, tc, xTp, wqT, wkT, wvT, wop, bqp, bkp, bvr, ones1, out, rcd):
    import concourse.bass as bass_mod

    ctx = ExitStack()
    with ctx:
        singles = ctx.enter_context(tc.tile_pool(name="singles", bufs=1))
        big = ctx.enter_context(tc.tile_pool(name="big", bufs=1))
        w_pool = ctx.enter_context(tc.tile_pool(name="wts", bufs=24))
        pt_pool = ctx.enter_context(tc.tile_pool(name="pt", bufs=4))
        stg_pool = ctx.enter_context(tc.tile_pool(name="stg", bufs=4))
        stgc_pool = ctx.enter_context(tc.tile_pool(name="stgc", bufs=4))
        bch_pool = ctx.enter_context(tc.tile_pool(name="bch", bufs=2))

        # ---- big persistent tiles ----
        XT = big.tile([128, 8 * RB], BF16, tag="xt", name="XT")
        # QQ/KK: pair-major free dim: col = a*2048 + pi, pi = w*1024+mp*128+tt
        # partitions: z*64 + dd (z = head parity within pair)
        QQ = big.tile([128, 2 * T], BF16, tag="qq", name="QQ")
        KK = big.tile([128, 2 * T], BF16, tag="kk", name="KK")
        # VN[l]: [128(tt), 16*65]; slab s = w*8 + mp_global; ones at col 65s+64
        VN = [big.tile([128, 16 * 65], BF16, tag=f"vn{l}", name=f"vn{l}")
              for l in range(HPC)]
        # ONP[a]: normalized O^T, pair-stacked: partition z*64+dd, free pi
        ONP = [big.tile([128, T], BF16, tag=f"onp{a}", name=f"onp{a}")
               for a in range(2)]
        WOP = [big.tile([128, DM], BF16, tag=f"wop{a}", name=f"wopp{a}")
               for a in range(2)]
        OU = big.tile([DH, 2 * DM], BF16, tag="ou", name="OU")  # per-unit O^T
        ON1 = big.tile([DH, DM], BF16, tag="on1", name="ON1")  # z=1 scratch
        rcp = big.tile([1, 2 * DM], BF16, tag="rcp", name="rcp")

        bq_sb = singles.tile([128, 8], F32, tag="bq", name="bq_sb")
        bk_sb = singles.tile([128, 8], F32, tag="bk", name="bk_sb")
        bv_sb = singles.tile([1, DM], F32R, tag="bv", name="bv_sb")
        ones_r = singles.tile([1, 128], F32R, tag="ones", name="ones_r")
        onescol = singles.tile([128, 16], BF16, tag="onescol", name="onescol")
        ones64 = singles.tile([1, 64], BF16, tag="ones64", name="ones64")
        nc.vector.memset(onescol, 1.0)
        nc.vector.memset(ones64, 1.0)

        # ---- initial DMAs ----
        # One descriptor lands on ONE of the 16 DMA engines (~40GB/s each),
        # so startup-critical tiles are split into several descriptors that
        # fan out round-robin; everything else is one descriptor per chunk,
        # all triggered up front (descriptors self-gate on semaphores).
        # Ring budget matters: each trigger is ~600ns of sequencer time.
        x3o = XT.rearrange("p (k c) -> p k c", c=RB)
        x3i = xTp.rearrange("p (k c) -> p k c", c=RB)
        W = {}
        for nm, src in (("q", wqT), ("k", wkT), ("v", wvT)):
            for p in range(8):
                W[(nm, p)] = w_pool.tile([128, DM], BF16, tag="w",
                                         name=f"w{nm}{p}")

        # x on the sync ring, wq on the scalar ring IN PARALLEL so the
        # first matmul's two operands land together (~3us after preamble)
        for p in range(8):
            nsplit = 2 if p < 2 else 1
            for h in range(nsplit):
                w = 128 // nsplit
                nc.sync.dma_start(out=x3o[w * h:w * (h + 1), p, :],
                                  in_=x3i[w * h:w * (h + 1), p, :])
        for p in range(8):
            nsplit = 4 if p == 0 else 2
            for h in range(nsplit):
                w = 128 // nsplit
                nc.scalar.dma_start(
                    out=W[("q", p)][w * h:w * (h + 1), :],
                    in_=wqT[p, w * h:w * (h + 1), :])
            if p == 1:
                # biases are needed by the first drain (~14us in), after
                # wq p0/p1 which gate the very first matmuls
                nc.scalar.dma_start(out=bq_sb, in_=bqp)
                nc.scalar.dma_start(out=bk_sb, in_=bkp)
                nc.scalar.dma_start(out=bv_sb, in_=bvr)
                nc.scalar.dma_start(out=ones_r, in_=ones1)
        # K and V both on scalar: the sync ring must stay clear for the
        # just-in-time Q/K drain DMAs
        for p in range(8):
            for h in range(2):
                nc.scalar.dma_start(out=W[("k", p)][64 * h:64 * (h + 1), :],
                                    in_=wkT[p, 64 * h:64 * (h + 1), :])
        for p in range(8):
            nc.scalar.dma_start(out=W[("v", p)], in_=wvT[p])

        # ---------- Stage A: QKV projections (interleaved halves) ----------
        def qk_drain(ps, m2, b_sb, dest):
            """Drain one Q/K psum m2-tile into QQ/KK pair layout + bias.
            All on DVE: ACT's in-order queue would serialize these behind
            each half's last matmul and starve the first exps."""
            src_e = ps[0:64, :].rearrange("p (a zz x) -> p zz a x", a=2, zz=2)
            src_o = ps[64:128, :].rearrange("p (a zz x) -> p zz a x", a=2, zz=2)
            d3 = dest.rearrange("p (a q) -> p a q", a=2)
            # z0w0: psum rows 0:64 of even-parity heads -> same partitions
            nc.vector.tensor_scalar(
                out=d3[0:64, :, 128 * m2:128 * m2 + 128],
                in0=src_e[:, 0, :, :], scalar1=b_sb[0:64, m2:m2 + 1],
                scalar2=None, op0=ADD)
            # z1w1: psum rows 64:128 of odd-parity heads -> same partitions
            nc.vector.tensor_scalar(
                out=d3[64:128, :, 1024 + 128 * m2:1024 + 128 * m2 + 128],
                in0=src_o[:, 1, :, :], scalar1=b_sb[64:128, m2:m2 + 1],
                scalar2=None, op0=ADD)
            # cross-partition pieces via stg + SBUF->SBUF DMA:
            # (z1,w0) = psum rows 0:64 of odd-parity heads -> dest rows 64:128
            # (z0,w1) = psum rows 64:128 of even-parity heads -> dest rows 0:64
            stg = stg_pool.tile([128, 256], BF16, tag="stg", name=f"stg{m2}")
            stg3 = stg.rearrange("p (a x) -> p a x", a=2)
            nc.vector.tensor_scalar(
                out=stg3[0:64], in0=src_e[:, 1, :, :],
                scalar1=b_sb[0:64, m2:m2 + 1], scalar2=None, op0=ADD)
            nc.vector.tensor_scalar(
                out=stg3[64:128], in0=src_o[:, 0, :, :],
                scalar1=b_sb[64:128, m2:m2 + 1], scalar2=None, op0=ADD)
            # cross DMAs ride the sync HWDGE ring just-in-time (the gpsimd
            # SWDGE queue serializes at ~1us/trigger and delayed QQ/KK)
            # z1w0: dest[64:128, a*2048 + m2*128] <- stg rows 0:64
            nc.sync.dma_start(
                out=d3[64:128, :, 128 * m2:128 * m2 + 128], in_=stg3[0:64])
            # z0w1: dest[0:64, a*2048 + 1024 + m2*128] <- stg rows 64:128
            nc.sync.dma_start(
                out=d3[0:64, :, 1024 + 128 * m2:1024 + 128 * m2 + 128],
                in_=stg3[64:128])

        def v_drain(ps, l, cc):
            vdst = VN[l].rearrange("p (w s8 e) -> p s8 w e", w=2, e=65)
            src = ps.rearrange("p (mp w e) -> p mp w e", mp=4, w=2)
            nc.vector.tensor_copy(vdst[:, 4 * cc:4 * cc + 4, :, 0:64], src)
            if cc == 1:
                v3 = VN[l].rearrange("p (s e) -> p s e", e=65)
                nc.vector.tensor_copy(
                    v3[:, :, 64:65], onescol.rearrange("p (s o) -> p s o", o=1))

        with tc.tile_pool(name="psA", bufs=1, space="PSUM") as psA:
            psq = [psA.tile([128, RB], F32, tag=f"A{i}", name=f"psq{i}")
                   for i in range(8)]
            for pname, b_sb, dest in (("q", bq_sb, QQ), ("k", bk_sb, KK)):
                for half in range(2):
                    m2s = range(4 * half, 4 * half + 4)
                    for p in range(8):
                        for m2 in m2s:
                            nc.tensor.matmul(
                                psq[m2][:],
                                W[(pname, p)][:, 128 * m2:128 * (m2 + 1)],
                                x3o[:, p, :],
                                start=(p == 0), stop=(p == 7))
                    # drain this half while the other half's matmuls stream
                    for m2 in m2s:
                        qk_drain(psq[m2], m2, b_sb, dest)
            # V head 0 only; heads 1-3 run as fillers inside attention
            psv = {}
            for cc in range(2):
                psv[(0, cc)] = psA.tile(
                    [128, 512], F32, tag=f"A{cc * 2}", name=f"psv0_{cc}")
            for p in range(8):
                for cc in range(2):
                    nc.tensor.matmul(
                        psv[(0, cc)][:],
                        x3o[:, p, 0:128],
                        W[("v", p)][:, 512 * cc:512 * (cc + 1)],
                        start=(p == 0), stop=False)
            for cc in range(2):
                nc.tensor.matmul(
                    psv[(0, cc)][:], ones_r[0:1, 0:128],
                    bv_sb[0:1, 512 * cc:512 * (cc + 1)],
                    start=False, stop=True)
                v_drain(psv[(0, cc)], 0, cc)

        # WOP is first needed by the pair-1 output projection; load it late
        # so it never delays the startup-critical x/weight descriptors.
        nc.sync.dma_start(out=WOP[0], in_=wop[0])
        nc.scalar.dma_start(out=WOP[1], in_=wop[1])

        # ---------- Stage B: flat software-pipelined attention ----------
        # 128 global slab-cycles (8 units x 16 slabs). Per cycle gs:
        #   ACT: exp(gs)            (paced back-to-back, the phase pacer)
        #   PE:  S(gs+2), PV(gs-1), [<=1 filler]
        # S leads its exp by 2 cycles and PV lags its exp by 1, so every PE
        # gate (psS WAR on exp(gs), pt RAW on exp(gs-1)) cleared >=1 cycle
        # before execution: the PE never micro-stalls (keeps the 2.4GHz
        # p-state), and the ACT stream never waits on S.
        fillers = []  # queue of closures, each emitting ~1-2 PE matmuls

        def make_v_fillers(psC, l):
            tiles = {}

            def mk(cc, p):
                def go():
                    if p == 0:
                        tiles[cc] = psC.tile(
                            [128, 512], F32, tag="c", name=f"psv{l}_{cc}")
                    nc.tensor.matmul(
                        tiles[cc][:],
                        x3o[:, p, 128 * l:128 * (l + 1)],
                        W[("v", p)][:, 512 * cc:512 * (cc + 1)],
                        start=(p == 0), stop=False)
                    if p == 7:
                        nc.tensor.matmul(
                            tiles[cc][:], ones_r[0:1, 0:128],
                            bv_sb[0:1, 512 * cc:512 * (cc + 1)],
                            start=False, stop=True)
                        v_drain(tiles[cc], l, cc)
                return go

            for cc in range(2):
                for p in range(8):
                    fillers.append(mk(cc, p))

        def make_outproj_fillers(psC, sqs, tail=False):
            def mk(sq, cc, use_act):
                def go():
                    oc = psC.tile([128, 512], F32, tag="c",
                                  name=f"oc{sq}_{cc}")
                    for a in range(2):
                        nc.tensor.matmul(
                            oc[:],
                            ONP[a][:, 128 * sq:128 * sq + 128],
                            WOP[a][:, 512 * cc:512 * (cc + 1)],
                            start=(a == 0), stop=(a == 1))
                    stgc = stgc_pool.tile([128, 512], BF16, tag="stgc",
                                          name=f"stgc{sq}_{cc}")
                    if use_act:  # ACT is idle in the tail; split the drain
                        nc.scalar.activation(
                            stgc[:], oc[:], mybir.ActivationFunctionType.Copy)
                    else:
                        nc.vector.tensor_copy(stgc[:], oc[:])
                    # never the scalar ring mid-attention (ACT sequencer)
                    eng = nc.sync if (sq + cc) % 2 == 0 else nc.gpsimd
                    eng.dma_start(
                        out=out[128 * sq:128 * (sq + 1),
                                512 * cc:512 * (cc + 1)],
                        in_=stgc[:])
                return go

            for sq in sqs:
                for cc in range(2):
                    fillers.append(mk(sq, cc, tail and (sq + cc) % 2 == 1))

        with (
            tc.tile_pool(name="psS", bufs=2, space="PSUM") as psS,
            tc.tile_pool(name="psO", bufs=1, space="PSUM") as psO,
            tc.tile_pool(name="psC", bufs=2, space="PSUM") as psC,
        ):
            AQ = QQ.rearrange("p (a q) -> p a q", a=2)
            AK = KK.rearrange("p (a q) -> p a q", a=2)
            # V heads 1-3 projections become the early fillers
            for l in (1, 2, 3):
                make_v_fillers(psC, l)
            # unit order: head-major for V-filler readiness; (3,*) before
            # the last unit so only z=0 (2,1) gates the tail
            units = [(0, 0), (0, 1), (1, 0), (1, 1),
                     (2, 0), (3, 0), (3, 1), (2, 1)]
            NU = len(units)
            sp_t = {}    # gs -> psS tile
            pt_t = {}    # gs -> exp output tile
            op_t = {}    # unit -> psO tile
            OUR = big.tile([65, 2 * DM], BF16, tag="our", name="OUR")

            def emit_S(gs):
                u, s = divmod(gs, 16)
                l, hf = units[u]
                a, z = l // 2, l % 2
                zs = slice(64 * z, 64 * z + 64)
                sp = psS.tile([128, 1024], F32, tag="s", name=f"ps{gs}")
                sp_t[gs] = sp
                for qc in range(2):
                    nc.tensor.matmul(
                        sp[:, 512 * qc:512 * (qc + 1)],
                        AK[zs, a, 128 * s:128 * s + 128],
                        AQ[zs, a, 1024 * hf + 512 * qc:
                           1024 * hf + 512 * qc + 512],
                        start=True, stop=True)

            def emit_exp(gs):
                pt = pt_pool.tile([128, 1024], BF16, tag="pt", name=f"pt{gs}")
                pt_t[gs] = pt
                nc.scalar.activation(pt[:], sp_t.pop(gs)[:], EXPF, scale=0.125)

            def emit_PV(gs):
                u, s = divmod(gs, 16)
                l, hf = units[u]
                if s == 0:
                    op_t[u] = psO.tile([65, 1024], F32, tag="o", name=f"po{u}")
                op = op_t[u]
                pt = pt_t.pop(gs)
                for qc in range(2):
                    nc.tensor.matmul(
                        op[:, 512 * qc:512 * qc + 512],
                        VN[l][:, 65 * s:65 * s + 65],
                        pt[:, 512 * qc:512 * (qc + 1)],
                        start=(s == 0), stop=(s == 15))
                if s == 15:
                    emit_norm(u)

            def emit_norm(u):
                l, hf = units[u]
                a, z = l // 2, l % 2
                op = op_t.pop(u)
                cols = slice(1024 * hf, 1024 * (hf + 1))
                # bank-wise [65,512] drains so the next unit's first PV
                # (one cycle later) finds the psO banks already free
                for qc in range(2):
                    nc.vector.tensor_copy(
                        OUR[:, 1024 * hf + 512 * qc:1024 * hf + 512 * (qc + 1)],
                        op[:, 512 * qc:512 * (qc + 1)])
                rc = OUR[64:65, cols]
                scr = stg_pool.tile([128, 8], BF16, tag="scr", name=f"sc{u}")
                scr2 = stg_pool.tile([128, 8], BF16, tag="sc2", name=f"s2{u}")
                # the last two units gate the tail: use the low-latency sync
                # HWDGE ring for their chain instead of SWDGE
                dring = nc.sync if u >= 6 else nc.gpsimd
                dring.dma_start(
                    out=scr, in_=rc.rearrange("o (r c) -> o r c", c=8))
                with nc.allow_low_precision(
                        reason="softmax denom reciprocal in bf16: ~0.4% "
                               "rel, well inside the 2e-2 gate"):
                    nc.vector.reciprocal(scr2, scr)
                if u == NU - 1:
                    # last unit gates the tail and the PE is idle: gather
                    # the reciprocal row to SBUF and broadcast with a K=1
                    # matmul instead of the slower DRAM round-trip
                    rc0 = stg_pool.tile([1, DM], BF16, tag="rc0", name="rc0")
                    nc.sync.dma_start(out=rc0, in_=scr2)
                    for qc in range(2):
                        bc = psC.tile([128, 512], F32, tag="c",
                                      name=f"bcl{qc}")
                        nc.tensor.matmul(
                            bc[0:64, :], ones64[0:1, :],
                            rc0[0:1, 512 * qc:512 * (qc + 1)],
                            start=True, stop=True)
                        nc.vector.tensor_mul(
                            ONP[a][0:64, 1024 * hf + 512 * qc:
                                   1024 * hf + 512 * (qc + 1)],
                            OUR[0:64, 1024 * hf + 512 * qc:
                                1024 * hf + 512 * (qc + 1)],
                            bc[0:64, :])
                    return
                dring.dma_start(out=rcd[u % 2], in_=scr2)
                bch = bch_pool.tile([64, DM], BF16, tag="b", name=f"bch{u}")
                bcast_ap = bass_mod.AP(
                    tensor=rcd[u % 2].tensor, offset=rcd[u % 2].offset,
                    ap=[[0, 64], list(rcd[u % 2].ap[-1])])
                dring.dma_start(out=bch, in_=bcast_ap)
                if z == 0:
                    nc.vector.tensor_mul(
                        ONP[a][0:64, cols], OUR[0:64, cols], bch[:])
                else:
                    # DVE cannot cross partitions: multiply into a base-0
                    # scratch, then DMA up to partitions 64-127 of ONP.
                    nc.vector.tensor_mul(ON1[:], OUR[0:64, cols], bch[:])
                    dring.dma_start(out=ONP[a][64:128, cols], in_=ON1[:])

            NG = 16 * NU
            emit_S(0)
            emit_S(1)
            for gs in range(NG):
                if gs == 106:
                    # sq<8 outproj: inputs complete once unit 5's normalize
                    # chain (~8us of DMA latency) lands
                    make_outproj_fillers(psC, range(8))
                emit_exp(gs)
                if gs + 2 < NG:
                    emit_S(gs + 2)
                if gs >= 2:
                    # PV lags its exp by 2 cycles so its pt-RAW semaphore
                    # landed a full cycle ago: no ACT->PE wait on PV
                    emit_PV(gs - 2)
                if fillers:
                    fillers.pop(0)()
            emit_PV(NG - 2)
            emit_PV(NG - 1)
            # tail: sq>=8 outproj (needs the last two units)
            make_outproj_fillers(psC, range(8, 16), tail=True)
            while fillers:
                fillers.pop(0)()


_NC_CACHE = None


def _get_program():
    global _NC_CACHE
    if _NC_CACHE is None:
        _NC_CACHE = build_program()
    return _NC_CACHE


def _prep_host(x, wq, bq, wk, bk, wv, bv, wo, bo, cos, sin):
    f32 = np.float32
    bf = ml_dtypes.bfloat16
    x = np.asarray(x, f32)
    wq, wk, wv, wo = (np.asarray(a, f32) for a in (wq, wk, wv, wo))
    bq, bk, bv, bo = (np.asarray(a, f32) for a in (bq, bk, bv, bo))
    cos, sin = np.asarray(cos, f32), np.asarray(sin, f32)

    # RoPE at fixed position T (reference bug, replicated): fold into weights.
    c_row = cos[T]
    s_row = sin[T]
    Cv = np.tile(c_row, H)
    Sv = np.tile(s_row, H)
    sgn = np.where(np.arange(DM) % 2 == 0, -1.0, 1.0).astype(f32)
    Ss = (sgn * Sv).astype(f32)
    swap = np.arange(DM) ^ 1

    wq_rot = Cv[:, None] * wq + Ss[:, None] * wq[swap, :]
    wk_rot = Cv[:, None] * wk + Ss[:, None] * wk[swap, :]
    bq_rot = Cv * bq + Ss * bq[swap]
    bk_rot = Cv * bk + Ss * bk[swap]

    wqTc = np.ascontiguousarray(wq_rot.T).reshape(8, 128, DM).astype(bf)
    wkTc = np.ascontiguousarray(wk_rot.T).reshape(8, 128, DM).astype(bf)
    wvTc = np.ascontiguousarray(wv.T).reshape(8, 128, DM).astype(bf)
    # bias per m2-tile partition: bqp[r, m2] = bq_rot[128*m2 + r]
    bqp = np.ascontiguousarray(bq_rot.reshape(8, 128).T).astype(f32)
    bkp = np.ascontiguousarray(bk_rot.reshape(8, 128).T).astype(f32)

    in_maps = []
    for i in range(N_CORES):
        b, j = i // 4, i % 4
        xT = x[b, RB * j:RB * (j + 1), :].T  # [1024, 512]
        xTp = np.ascontiguousarray(
            xT.reshape(8, 128, RB).transpose(1, 0, 2).reshape(128, 8 * RB)
        ).astype(bf)
        wopc = np.stack([
            np.ascontiguousarray(
                wo[:, 256 * j + 128 * a:256 * j + 128 * (a + 1)].T)
            for a in range(2)
        ]).astype(bf)
        in_maps.append({
            "xTp": xTp, "wqT": wqTc, "wkT": wkTc, "wvT": wvTc, "wop": wopc,
            "bqp": bqp, "bkp": bkp, "bvr": bv.reshape(1, DM),
            "ones1": np.ones((1, 128), f32),
        })
    return in_maps, bo


def kernel(x, wq, bq, wk, bk, wv, bv, wo, bo, cos, sin,
           _trace=False, _trace_kwargs=None):
    nc = _get_program()
    in_maps, bo_np = _prep_host(x, wq, bq, wk, bk, wv, bv, wo, bo, cos, sin)
    kw = {}
    if _trace:
        kw["trace"] = True
        if _trace_kwargs:
            kw.update(_trace_kwargs)
    res = run_bass_kernel_spmd(nc, in_maps, core_ids=list(range(N_CORES)), **kw)
    outf = np.zeros((B, T, DM), np.float32)
    for i in range(N_CORES):
        part = res.results[i]["out"].astype(np.float32)
        # rows arrive as pi = (w*8+mp)*128 + tt; t = tt*16 + 2*mp + w
        part = part.reshape(2, 8, 128, DM).transpose(2, 1, 0, 3).reshape(T, DM)
        outf[i // 4] += part
    outf += bo_np[None, None, :]
    kernel.last_results = res
    return outf
